# revision 59
# baseline (speedup 1.0000x reference)
"""Trainium2 Bass kernel for fused MultiHeadAttention + residual + LayerNorm.

Problem: query [4, 2048, 512] f32, H=8 heads (hd=64), fused QKV projection,
key-padding-mask softmax, attn @ V, residual add, LayerNorm over D=512.

Sharding: 8 cores = 4 batches x 2 query-halves. Each core handles one batch's
full K/V (T=2048) and 1024 query rows, so heads stay local and the output
LayerNorm needs no cross-core communication. K/V projection is duplicated
between the 2 cores sharing a batch; X^T columns are rotated per core so its
own query half sits at columns 0:Q (Q^T projects straight out of X^T, no
separate xq input, and attention is k-permutation-invariant with the mask
rotated to match).

K/Q projections and scores run in bf16 (fp32 PSUM accum) — fp8 K-proj was
measured offline at 2.6e-2 max error (score errors amplify through exp),
past the 2e-2 budget, so K/Q must stay bf16; score matmuls are 2x512-col
(a single 1024-col moving matmul fails the ISA check). The V projection,
attention weights P = exp(S/8 - ln64) and V run fp8e4 so those matmuls use
DoubleRow perf mode: one pass contracts TWO 128-row slices at the same
1 col/cycle -> half the PE time. fp8 operands are quantized on the host
straight from f32 (f32->bf16->fp8 double rounding costs 1.5x in max error).
The 1/64 P scaling keeps exp() inside e4m3 range for the 9-sigma score
tails (max raw score 71.9); the denominator (ones-column 0 of each V head
group) scales identically so the softmax ratio is unaffected.

Softmax exp is the Scalar/ACT bottleneck (128 tiles x [128,1024], ~1.05us
each), so per head 2-3 tiles (more on late heads, 7 on the last) go to DVE
via the Schraudolph bit-trick
  fp8bits(exp(s)) ~= uint8(s*(8/ln2)*SCALE + 7.65 + maskbias)
(f32->uint8 convert is round-to-nearest saturating to [0,255] on HW, so the
exp underflow tail AND masked rows — addend -1e6 — clamp to +0.0; scores
never reach the bits>=120 inf/nan region). The uint8 tile is bitcast to
fp8e4 for the DoubleRow matmul. ~4% rms error on those P tiles, attenuated
~25x by the softmax-weighted average + f32 residual, keeps total error
~1.4e-2 < the 2e-2 budget. GpSimd cannot read PSUM so it takes SBUF-only
work (memsets, LayerNorm affine).

Per-core flow:
  X^T [512,2048] bf16, W^T [512,1536] bf16, X/W_v fp8 d-pair tensors
  K^T [512,2048] bf16, Q^T head-major zero-padded to K=128 contraction
  V8  8 pair-tiles [128,2,H,80] fp8  (col 0 of each head group = 1.0)
  S^T [128k,1024q] f32 PSUM per (head,k-tile) -> exp (ACT fp8 out / DVE
      u8 trick) -> P pair tiles [128,2,1024] fp8
  O^T [65,1024] f32 = [1|V_h].T @dr P^T accumulated over 8 k-pairs
  bf16 copy -> PE-transpose [65,128]->[128,65], DVE reciprocal(denom),
  fused multiply-add folds the residual in per head slice; the recombine's
  accum_out collects row-sums for the LayerNorm mean
  LayerNorm (stage-batched in 2 groups of 4 q-tiles): var = E[y^2]-mean^2
  with ACT Square+accum (one Exp->Square table swap after the last exp),
  one batched Sqrt, DVE normalize, DVE/GpSimd affine -> DMA out f32.

Scheduling (measured on HW): PE warm-up matmuls run during the initial DMA
wait (HAM clock gate); DMA issue order follows the dependency chain (wt K/Q
column slices -> xt q-half -> xt rest -> fp8 V operands -> deferred
K/Q blocks -> xres/LN params last); head 0 fuses score/V-proj/attention@V
per k-pair so av(0) starts as soon as exp pair 0 lands; every later
attention@V interleaves per k-pair with the next head's score tiles and
deferred projection chunks (block b must be complete before av(2b-1) which
emits its consumer scores); the last head runs attention@V qcn-outer so the
first query half's epilogue overlaps the second half.
"""

import numpy as np

B, T, D = 4, 2048, 512
H, HD = 8, 64
Q = T // 2          # query rows per core
NCORES = 8
KT = T // 128       # 16 k-tiles
KP = KT // 2        # 8 k-pairs (DoubleRow)
QT = Q // 128       # 8 q-tiles
DC = D // 128       # 4 contraction chunks
SCALE = 1.0 / np.sqrt(HD)  # 0.125
EPS = 1e-5
MASK_BIAS = -1e9
LNP = float(np.log(64.0))      # P scaled by 1/64: max raw score is 71.9
                               # (9 sigma tails), exp(71.9/8)/64 = 125 < 240
SCHRAU_A = 8.0 / np.log(2.0)   # fp8e4 bits per e-fold
SCHRAU_B = 7.65                # (7-6)*8 (exp bias 7, scale 2^-6) - 0.35 centering
VP = 80                        # fp8 V row pitch (65 used, 16B-aligned)

# exp engine split per head: DVE takes one middle pair (attention@V reaches
# it 3/8 through the head, when DVE's queue has drained — last-pair
# assignment stalled the PE at every head boundary). GpSimd cannot read
# PSUM, so it gets the SBUF-only LayerNorm work instead. Overridden to
# "all ACT" by test.py --sim (CoreSim's u8 convert wraps instead of
# saturating).
DVE_EXP = {h: (6, 12) if h < 4 else
           ((3, 6, 9, 12, 14) if h < 7 else (1, 3, 5, 7, 9, 11, 13))
           for h in range(H)}

_CACHE = {}


def _emit(nc, tc, tens):
    import contextlib

    import concourse.bass as bass
    from concourse import mybir
    from concourse.masks import make_identity

    f32 = mybir.dt.float32
    bf16 = mybir.dt.bfloat16
    f8 = mybir.dt.float8e4
    u8 = mybir.dt.uint8
    Alu = mybir.AluOpType
    Act = mybir.ActivationFunctionType
    DR = mybir.MatmulPerfMode.DoubleRow

    with contextlib.ExitStack() as stack:
        persist = stack.enter_context(tc.tile_pool(name="persist", bufs=1))
        small = stack.enter_context(tc.tile_pool(name="small", bufs=8))
        expp = stack.enter_context(tc.tile_pool(name="expp", bufs=KP + 4))
        otsbp = stack.enter_context(tc.tile_pool(name="otsbp", bufs=2))
        outp = stack.enter_context(tc.tile_pool(name="outp", bufs=2))
        pps = stack.enter_context(tc.tile_pool(name="pps", bufs=2, space="PSUM"))
        stp = stack.enter_context(tc.tile_pool(name="stp", bufs=2, space="PSUM"))
        scr = stack.enter_context(tc.tile_pool(name="scr", bufs=2, space="PSUM"))

        # ---- persistent tiles ----
        wt_sb = [persist.tile([128, 3 * D], bf16, name=f"wtsb{i}", tag=f"wtsb{i}")
                 for i in range(DC)]
        xt_sb = [persist.tile([128, T], bf16, name=f"xtsb{i}", tag=f"xtsb{i}")
                 for i in range(DC)]
        kt_sb = [persist.tile([128, T], bf16, name=f"ktsb{i}", tag=f"ktsb{i}")
                 for i in range(DC)]
        # Per-head Q^T padded to 128 contraction rows: rows (h%2)*64..+64 hold
        # Q_h, the other 64 rows stay zero. Keeps the score matmuls at K=128 —
        # K=64 matmuls don't register as PE activity for the HAM clock gate
        # and leave the whole attention phase throttled to 1.2 GHz.
        qt_pad = [persist.tile([128, Q], bf16, name=f"qtpad{h}", tag=f"qtpad{h}")
                  for h in range(H)]
        # X^T and W_v in fp8 d-pair layout for DoubleRow V-projection:
        # slice s of x8a holds X^T rows s*128..(s+1)*128
        x8p = [persist.tile([128, 2, T], f8, name=f"x8p{j}", tag=f"x8p{j}")
               for j in range(2)]
        wv8 = [persist.tile([128, 2, D], f8, name=f"wv8{j}", tag=f"wv8{j}")
               for j in range(2)]
        # V in fp8, k-pair major for DoubleRow: [k-part, pair-slice, head, col]
        # col 0 = 1.0 (denominator), cols 1:65 = V_h, 65:80 pad (16B stride).
        v8_sb = [persist.tile([128, 2, H, VP], f8, name=f"v8sb{p}",
                              tag=f"v8sb{p}") for p in range(KP)]
        oacc = [persist.tile([128, D], f32, name=f"oacc{q}", tag=f"oacc{q}")
                for q in range(QT)]
        rs_all = persist.tile([128, QT, H], f32, name="rs_all", tag="rs_all")
        ssq8 = persist.tile([128, QT], f32, name="ssq8", tag="ssq8")
        mean8 = persist.tile([128, QT], f32, name="mean8", tag="mean8")
        rstd8 = persist.tile([128, QT], f32, name="rstd8", tag="rstd8")
        xres_sb = persist.tile([128, QT, D], f32, name="xres_sb", tag="xres_sb")
        btr_sb = persist.tile([128, 12], f32, name="btr_sb", tag="btr_sb")
        maska_sb = persist.tile([128, KT], f32, name="maska_sb", tag="maska_sb")
        maskd_sb = persist.tile([128, KT], f32, name="maskd_sb", tag="maskd_sb")
        lnw_sb = persist.tile([128, D], f32, name="lnw_sb", tag="lnw_sb")
        lnb_sb = persist.tile([128, D], f32, name="lnb_sb", tag="lnb_sb")
        ident65 = persist.tile([HD + 1, HD + 1], bf16, name="ident65",
                               tag="ident65")

        # ---- input DMAs, in dependency-priority order: the first score
        # matmul needs K^T block 0 (wt K-columns + xt t-chunk 0) and Q^T
        # block 0, so those chunks land first ----
        rows = lambda i: slice(i * 128, (i + 1) * 128)
        wm_sb = persist.tile([128, 640], bf16, name="wm_sb", tag="wm_sb")
        nc.vector.memset(wm_sb, 0.5)
        # Critical-path loads split across four issuing queues (sync, scalar,
        # vector, gpsimd) so the per-engine semaphore chains run in parallel.
        # Priority: small bias/mask tiles, then the K-proj critical prefix
        # (wt K-cols + xt t-chunk 0), then xq/wt-Q for Q^T, then the rest.
        # scalar queue: only the small critical prefix (done by ~7us, then
        # free for exp); sync queue: everything else in dependency order.
        nc.scalar.dma_start(out=maskd_sb, in_=tens["maskd"][:])
        nc.scalar.dma_start(out=btr_sb, in_=tens["btr"][:])
        nc.sync.dma_start(out=maska_sb, in_=tens["maska"][:])
        for i in range(DC):     # wt K block-0 columns: unblock emit_kt(0)
            nc.scalar.dma_start(out=wt_sb[i][:, D:D + 128],
                                in_=tens["wt"][rows(i), D:D + 128])
        for i in range(DC):     # wt Q block-0 columns: unblock emit_qt(0)
            nc.scalar.dma_start(out=wt_sb[i][:, 0:128],
                                in_=tens["wt"][rows(i), 0:128])
        for i in range(2):      # xt q-half: gates Q^T (score moving operand),
            nc.sync.dma_start(out=xt_sb[i][:, 0:1024],
                              in_=tens["xt"][rows(i), 0:1024])
        for i in range(2, DC):  # split across the idle SWDGE ring
            nc.gpsimd.dma_start(out=xt_sb[i][:, 0:1024],
                                in_=tens["xt"][rows(i), 0:1024])
        for i in range(DC):     # rest of xt (scores k>=8, V-proj input)
            nc.sync.dma_start(out=xt_sb[i][:, 1024:T],
                              in_=tens["xt"][rows(i), 1024:T])
        for j in range(2):      # V-proj fp8 inputs (emit_v runs after scores 0)
            nc.sync.dma_start(out=wv8[j], in_=tens["wv8"][j])
            nc.sync.dma_start(out=x8p[j][:, :, 0:T // 2],
                              in_=tens["x8p"][j, :, :, 0:T // 2])
            nc.sync.dma_start(out=x8p[j][:, :, T // 2:T],
                              in_=tens["x8p"][j, :, :, T // 2:T])
        for i in range(DC):     # remaining K/Q projection weights (head >= 2)
            nc.sync.dma_start(out=wt_sb[i][:, D + 128:2 * D],
                              in_=tens["wt"][rows(i), D + 128:2 * D])
        for i in range(DC):
            nc.sync.dma_start(out=wt_sb[i][:, 128:D],
                              in_=tens["wt"][rows(i), 128:D])
        # residual + LN params: only needed from the first recombine (~45us),
        # queued last on sync so their transfers don't eat startup bandwidth
        for q in range(QT):
            nc.sync.dma_start(out=xres_sb[:, q, :],
                              in_=tens["xres"][q * 128:(q + 1) * 128, :])
        for dst, key in ((lnw_sb, "lnw"), (lnb_sb, "lnb")):
            src_ap = tens[key][:]
            ap = bass.AP(tensor=src_ap.tensor, offset=src_ap.offset,
                         ap=[[0, 128]] + list(src_ap.ap))
            nc.sync.dma_start(out=dst, in_=ap)

        def bcast_row(dst, src_handle):
            src = src_handle[:]
            ap = bass.AP(tensor=src.tensor, offset=src.offset,
                         ap=[[0, 128]] + list(src.ap))
            nc.sync.dma_start(out=dst, in_=ap)

        for h in range(H):
            z0 = 64 * (1 - (h % 2))
            (nc.vector if h < 2 else nc.gpsimd).memset(
                qt_pad[h][z0:z0 + HD, :], 0.0)
        for p in range(KP):
            nc.gpsimd.memset(v8_sb[p][:, :, :, 0:1], 1.0)
        make_identity(nc, ident65)

        # ---- PE warm-up: K=128 matmuls with no data deps run during the
        # initial DMA wait so the HAM clock gate is already open (2.4 GHz)
        # when the projections start. The result is never used.
        wmps = stp.tile([128, Q], f32, name="wmps", tag="st")
        for i in range(10):
            nc.tensor.matmul(wmps[:, 0:512], wm_sb[:, 0:128],
                             wm_sb[:, 128:640], start=True, stop=True)
        wm_out = small.tile([128, 1], f32, name="wm_out", tag="wm_out")
        nc.vector.tensor_copy(out=wm_out, in_=wmps[:, 0:1])


        # ---- projection emitters ----
        def kt_chunk(i, tcn):
            ps = pps.tile([128, 512], f32, name="kps", tag="pps")
            for dc in range(DC):
                nc.tensor.matmul(
                    ps, wt_sb[dc][:, D + i * 128: D + (i + 1) * 128],
                    xt_sb[dc][:, tcn * 512:(tcn + 1) * 512],
                    start=(dc == 0), stop=(dc == DC - 1))
            nc.vector.tensor_scalar_add(
                out=kt_sb[i][:, tcn * 512:(tcn + 1) * 512],
                in0=ps, scalar1=btr_sb[:, 4 + i:5 + i])

        def emit_kt(i):
            for tcn in range(T // 512):
                kt_chunk(i, tcn)

        def qt_chunk(i, qcn):
            ps = pps.tile([128, 512], f32, name="qps", tag="pps")
            for dc in range(DC):
                nc.tensor.matmul(
                    ps, wt_sb[dc][:, i * 128:(i + 1) * 128],
                    xt_sb[dc][:, qcn * 512:(qcn + 1) * 512],
                    start=(dc == 0), stop=(dc == DC - 1))
            for j in range(2):
                r0 = j * HD
                nc.vector.tensor_scalar_add(
                    out=qt_pad[2 * i + j][r0:r0 + HD,
                                          qcn * 512:(qcn + 1) * 512],
                    in0=ps[r0:r0 + HD, :],
                    scalar1=btr_sb[r0:r0 + HD, i:i + 1])

        def emit_qt(i):
            for qcn in range(Q // 512):
                qt_chunk(i, qcn)

        def emit_v(k):
            # fp8 DoubleRow projection (2 matmuls contract all 512 d-rows).
            # V-bias is folded into xres host-side (attn-out = sum P (v+bv)
            # / sum P = attn + bv), so the copy is a pure PSUM->fp8 convert.
            ps = pps.tile([128, 512], f32, name="vps", tag="pps")
            for j in range(2):
                nc.tensor.matmul(
                    ps, x8p[j][:, :, k * 128:(k + 1) * 128], wv8[j][:],
                    start=(j == 0), stop=(j == 1), perf_mode=DR)
            nc.vector.tensor_copy(
                out=v8_sb[k // 2][:, k % 2, :, 1:HD + 1],
                in_=ps.rearrange("p (h d) -> p h d", h=H))

        # ---- LayerNorm epilogue: stage-batched across all 8 q-tiles.
        # var = E[y^2] - mean^2 (sums accumulated on DVE during the last
        # head), one batched ACT Sqrt = exactly one activation-table swap
        # in the whole kernel tail. ----
        def emit_ln_all(qgroup):
            # stats for 4 q-tiles (group A can start while the second half
            # of the last head's attention@V is still running)
            g = slice(qgroup * 4, qgroup * 4 + 4)
            rowsum4 = small.tile([128, 4], f32, name="rowsum4",
                                 tag="rowsum4")
            nc.vector.reduce_sum(out=rowsum4, in_=rs_all[:, g, :],
                                 axis=mybir.AxisListType.X)
            nc.vector.tensor_scalar_mul(out=mean8[:, g], in0=rowsum4,
                                        scalar1=1.0 / D)
            msq = small.tile([128, 4], f32, name="msq", tag="msq")
            nc.vector.tensor_tensor(out=msq, in0=mean8[:, g],
                                    in1=mean8[:, g], op=Alu.mult)
            var4 = small.tile([128, 4], f32, name="var4", tag="var4")
            nc.vector.tensor_scalar(out=var4, in0=ssq8[:, g],
                                    scalar1=1.0 / D, scalar2=EPS,
                                    op0=Alu.mult, op1=Alu.add)
            varc = small.tile([128, 4], f32, name="varc", tag="varc")
            nc.vector.tensor_tensor(out=varc, in0=var4, in1=msq,
                                    op=Alu.subtract)
            sd4 = small.tile([128, 4], f32, name="sd4", tag="sd4")
            nc.scalar.activation(out=sd4, in_=varc, func=Act.Sqrt)
            nc.vector.reciprocal(out=rstd8[:, g], in_=sd4)
            pool_q = {2, 5, 7}  # GpSimd affine tiles, emitted first so the
            order = sorted(range(qgroup * 4, qgroup * 4 + 4),
                           key=lambda q: q not in pool_q)  # slow engine starts early
            for q in order:
                yn = outp.tile([128, D], f32, name="yn", tag="yn")
                nc.vector.tensor_scalar(
                    out=yn, in0=oacc[q], scalar1=mean8[:, q:q + 1],
                    scalar2=rstd8[:, q:q + 1],
                    op0=Alu.subtract, op1=Alu.mult)
                eng = nc.gpsimd if q in pool_q else nc.vector
                yw = outp.tile([128, D], f32, name="yw", tag="yw")
                eng.tensor_tensor(out=yw, in0=yn, in1=lnw_sb, op=Alu.mult)
                yo = outp.tile([128, D], f32, name="yo", tag="yo")
                eng.tensor_tensor(out=yo, in0=yw, in1=lnb_sb, op=Alu.add)
                nc.sync.dma_start(out=tens["out"][q * 128:(q + 1) * 128, :],
                                  in_=yo)

        # ---- attention head emitters ----
        head_pairs = {}

        def epilogue_q(h, otsb, q):
            tp = pps.tile([128, HD + 1], bf16, name="tp", tag="pps")
            nc.tensor.transpose(
                tp, otsb[q // 4][:, (q % 4) * 128:(q % 4 + 1) * 128],
                ident65)
            rec = small.tile([128, 1], f32, name="rec", tag="rec")
            nc.vector.reciprocal(out=rec, in_=tp[:, 0:1])
            nc.vector.scalar_tensor_tensor(
                out=oacc[q][:, h * HD:(h + 1) * HD],
                in0=tp[:, 1:HD + 1], scalar=rec, op0=Alu.mult,
                in1=xres_sb[:, q, h * HD:(h + 1) * HD], op1=Alu.add,
                accum_out=rs_all[:, q, h:h + 1])
            if h == H - 1:
                # sum of squares for LayerNorm variance (E[y^2] - mean^2):
                # ACT Square needs no mean, and the ACT queue orders all 8
                # after the last exp, so the Exp->Square table swap happens
                # exactly once
                sqs = outp.tile([128, D], f32, name="sqs", tag="sqs")
                nc.scalar.activation(out=sqs, in_=oacc[q], func=Act.Square,
                                     accum_out=ssq8[:, q:q + 1])

        def emit_score_tile(h, k, pairs):
            """Scores for one k-tile + engine-split exp into pair tile."""
            blk = h // 2
            st = stp.tile([128, Q], f32, name="st", tag="st")
            for qcn in range(Q // 512):
                nc.tensor.matmul(
                    st[:, qcn * 512:(qcn + 1) * 512],
                    kt_sb[blk][:, k * 128:(k + 1) * 128],
                    qt_pad[h][:, qcn * 512:(qcn + 1) * 512],
                    start=None, stop=None)
            if k % 2 == 0:
                pairs.append(expp.tile([128, 2, Q], f8, name="ppair",
                                       tag="ppair"))
            pt = pairs[k // 2]
            if k not in DVE_EXP[h]:
                nc.scalar.activation(out=pt[:, k % 2, :], in_=st,
                                     func=Act.Exp,
                                     bias=maska_sb[:, k:k + 1], scale=SCALE)
            else:
                nc.vector.tensor_scalar(
                    out=pt[:, k % 2, :].bitcast(u8), in0=st,
                    scalar1=float(SCALE * SCHRAU_A),
                    scalar2=maskd_sb[:, k:k + 1],
                    op0=Alu.mult, op1=Alu.add)

        def emit_scores(h):
            pairs = head_pairs[h] = []
            for k in range(KT):
                emit_score_tile(h, k, pairs)

        def av_pair(h, ots, kp, qcn_range=(0, 1)):
            pairs = head_pairs[h]
            for qcn in qcn_range:
                nc.tensor.matmul(
                    ots[qcn], v8_sb[kp][:, :, h, 0:HD + 1],
                    pairs[kp][:, :, qcn * 512:(qcn + 1) * 512],
                    start=(kp == 0), stop=(kp == KP - 1),
                    perf_mode=DR)

        def emit_av(h, inter_with=None, ots=None, done_pairs=0, extra=()):
            pairs = head_pairs[h]
            extra = list(extra)
            # O^T[1+d, q] accumulated over k-pairs via fp8 DoubleRow; V_h
            # stationary so its weight load hides behind the 512-col moving
            # stream. Interleaved per k-pair with the NEXT head's score/exp
            # emission (and any deferred projection chunks) so ACT/PE never
            # starve behind a dense attention@V block.
            otsb = [otsbp.tile([HD + 1, 512], bf16, name=f"otsb{qcn}",
                               tag=f"otsb{qcn}") for qcn in range(Q // 512)]
            if ots is None:
                ots = [scr.tile([HD + 1, 512], f32, name=f"ot{qcn}", tag="ot")
                       for qcn in range(Q // 512)]
            if h != H - 1:
                if inter_with is not None:
                    npairs = head_pairs[inter_with] = []
                for kp in range(done_pairs, KP):
                    av_pair(h, ots, kp)
                    if inter_with is not None:
                        emit_score_tile(inter_with, 2 * kp, npairs)
                        emit_score_tile(inter_with, 2 * kp + 1, npairs)
                    if extra:
                        extra.pop(0)()
                for qcn in range(Q // 512):
                    nc.vector.tensor_copy(out=otsb[qcn], in_=ots[qcn])
                for q in range(QT):
                    epilogue_q(h, otsb, q)
            else:
                # last head: qcn-outer so the first query half finishes 8
                # matmuls early and its epilogue (transpose/recombine/ssq)
                # overlaps the second half's attention@V
                for qcn in range(Q // 512):
                    for kp in range(KP):
                        av_pair(h, ots, kp, qcn_range=(qcn,))
                    nc.vector.tensor_copy(out=otsb[qcn], in_=ots[qcn])
                    for q in range(qcn * 4, qcn * 4 + 4):
                        epilogue_q(h, otsb, q)
                    emit_ln_all(qcn)

        # ---- emission: block-0 projections and head 0's scores first (exp
        # stream starts before V-proj); each attention@V interleaves per
        # k-pair with the next head's scores so ACT stays fed ----
        emit_kt(0)
        emit_qt(0)
        pairs0 = head_pairs[0] = []
        ots0 = [scr.tile([HD + 1, 512], f32, name=f"ot{qcn}", tag="ot")
                for qcn in range(Q // 512)]
        for kp in range(KP):
            emit_score_tile(0, 2 * kp, pairs0)
            emit_score_tile(0, 2 * kp + 1, pairs0)
            emit_v(2 * kp)
            emit_v(2 * kp + 1)
            av_pair(0, ots0, kp)
        # head 1 scores standalone (ACT-bound stretch: deferred block-1
        # projections slot between score tiles without starving exp)
        from functools import partial
        blk1 = [partial(kt_chunk, 1, t) for t in range(4)] + [
            partial(qt_chunk, 1, c) for c in range(2)]
        pairs1 = head_pairs[1] = []
        for k in range(KT):
            emit_score_tile(1, k, pairs1)
            if k % 3 == 2 and blk1:
                blk1.pop(0)()
        while blk1:
            blk1.pop(0)()
        emit_av(0, ots=ots0, done_pairs=KP)
        blk2 = [partial(kt_chunk, 2, t) for t in range(4)] + [
            partial(qt_chunk, 2, c) for c in range(2)]
        emit_av(1, inter_with=2, extra=blk2[:3])
        emit_av(2, inter_with=3, extra=blk2[3:])
        blk3 = [partial(kt_chunk, 3, t) for t in range(4)] + [
            partial(qt_chunk, 3, c) for c in range(2)]
        emit_av(3, inter_with=4, extra=blk3[:3])
        emit_av(4, inter_with=5, extra=blk3[3:])
        emit_av(5, inter_with=6)
        emit_av(6, inter_with=7)
        emit_av(H - 1)

        # (residual + LayerNorm is emitted per q-tile from the last head)


def _build():
    import concourse.bacc as bacc
    import concourse.tile as tile
    from concourse import mybir

    f32 = mybir.dt.float32
    bf16 = mybir.dt.bfloat16
    nc = bacc.Bacc("TRN2", target_bir_lowering=False, debug=False)

    tens = {
        "xt": nc.dram_tensor("xt", [D, T], bf16, kind="ExternalInput"),
        "xres": nc.dram_tensor("xres", [Q, D], f32, kind="ExternalInput"),
        "wt": nc.dram_tensor("wt", [D, 3 * D], bf16, kind="ExternalInput"),
        "x8p": nc.dram_tensor("x8p", [2, 128, 2, T], mybir.dt.float8e4,
                              kind="ExternalInput"),
        "wv8": nc.dram_tensor("wv8", [2, 128, 2, D], mybir.dt.float8e4,
                              kind="ExternalInput"),
        "btr": nc.dram_tensor("btr", [128, 12], f32, kind="ExternalInput"),
        "maska": nc.dram_tensor("maska", [128, KT], f32, kind="ExternalInput"),
        "maskd": nc.dram_tensor("maskd", [128, KT], f32, kind="ExternalInput"),
        "lnw": nc.dram_tensor("lnw", [D], f32, kind="ExternalInput"),
        "lnb": nc.dram_tensor("lnb", [D], f32, kind="ExternalInput"),
        "out": nc.dram_tensor("out", [Q, D], f32, kind="ExternalOutput"),
    }

    with tile.TileContext(nc) as tc:
        _emit(nc, tc, tens)
    nc.compile()
    return nc


def make_in_maps(query, key_mask, in_proj_weight, in_proj_bias, ln_weight,
                 ln_bias):
    import ml_dtypes

    bf = ml_dtypes.bfloat16
    query = np.asarray(query, dtype=np.float32)
    key_mask = np.asarray(key_mask)
    w = np.asarray(in_proj_weight, dtype=np.float32)
    b = np.asarray(in_proj_bias, dtype=np.float32)
    lnw = np.asarray(ln_weight, dtype=np.float32)
    lnb = np.asarray(ln_bias, dtype=np.float32)

    wt = np.ascontiguousarray(w.T).astype(bf)
    btr = np.ascontiguousarray(b.reshape(12, 128).T)
    bv = b[2 * D:3 * D]  # folded into xres: attn-out(v+bv) = attn-out(v)+bv
    in_maps = []
    for c in range(NCORES):
        bi, half = c // 2, c % 2
        xb = query[bi]
        # k-columns reordered so this core's query half sits at 0:Q — the
        # Q-projection then reads xt directly (no separate xq input) and
        # attention is permutation-invariant over k as long as the mask
        # follows the same order.
        perm = (np.r_[Q:T, 0:Q] if half else np.arange(T))
        xbt = np.ascontiguousarray(xb.T[:, perm]).astype(bf)
        # fp8 operands quantized straight from f32: rounding f32->bf16->fp8
        # instead costs 1.5x in final max-error (boundary double rounding)
        f8 = ml_dtypes.float8_e4m3
        xbt8 = xb.T[:, perm].astype(f8)
        x8p = np.ascontiguousarray(
            xbt8.reshape(2, 2, 128, T).transpose(0, 2, 1, 3))
        wv8 = np.ascontiguousarray(
            w[2 * D:3 * D].T.astype(f8)
            .reshape(2, 2, 128, D).transpose(0, 2, 1, 3))
        km = key_mask[bi][perm]
        maskb = np.where(km, np.float32(MASK_BIAS), np.float32(0.0))
        maska = (maskb - LNP).astype(np.float32).reshape(KT, 128).T
        maskd = np.where(km, np.float32(-1e6),
                         np.float32(SCHRAU_B)).reshape(KT, 128).T
        in_maps.append({
            "xt": xbt,
            "xres": np.ascontiguousarray(xb[half * Q:(half + 1) * Q]
                                         + bv[None, :]),
            "wt": wt,
            "x8p": x8p,
            "wv8": wv8,
            "btr": btr,
            "maska": np.ascontiguousarray(maska),
            "maskd": np.ascontiguousarray(maskd),
            "lnw": lnw,
            "lnb": lnb,
        })
    return in_maps


def assemble(results):
    out = np.empty((B, T, D), dtype=np.float32)
    for c in range(NCORES):
        bi, half = c // 2, c % 2
        out[bi, half * Q:(half + 1) * Q] = results[c]["out"]
    return out


def get_nc():
    if "nc" not in _CACHE:
        _CACHE["nc"] = _build()
    return _CACHE["nc"]


def kernel(query, key_mask, in_proj_weight, in_proj_bias, ln_weight, ln_bias):
    from concourse.bass_utils import run_bass_kernel_spmd

    nc = get_nc()
    in_maps = make_in_maps(query, key_mask, in_proj_weight, in_proj_bias,
                           ln_weight, ln_bias)
    res = run_bass_kernel_spmd(nc, in_maps, core_ids=list(range(NCORES)))
    return assemble(res.results)
